# revision 10
# baseline (speedup 1.0000x reference)
"""Trainium2 Bass kernel for BlockDiagonalAggregator (moe_routing).

Computes, for each batch row b:
    logit[b,k] = dot(keys[sigma[b,k]], h[b,k,:])   (masked -inf where sigma==64)
    alpha      = softmax_k(logit)
    out[b,:]   = sum_k alpha[b,k] * h[b,k,:]

Distribution: data-parallel over B across 8 NeuronCores (512 rows each),
keys replicated, no collectives (per the data-parallel sharding hint).

Per-core algorithm (single streaming pass over h):
  - chunk = 128 (b,k)-slots = 2 full batch rows (K=64); macro = 16 chunks.
  - w gather via one-hot matmul: PE computes w = onehotT.T @ keys (bf16,
    one-hot built host-side from sigma).
  - logits via one fused DVE scalar_tensor_tensor (out=(h*1)*w, accum=sum_d).
  - e = exp(logit + pen) on ACT (pen = -1e9 for unassigned slots -> e = 0,
    matching the reference's mask+softmax; no max-subtraction needed since
    keys std 0.01 keeps |logit| < ~2).
  - e written into per-chunk-index persistent block-column stationaries
    E32[c] (128 x 32, fp32r): column 2c rows 0:64, column 2c+1 rows 64:128.
    Complementary cells stay zero (zero-filled once at start by DMA).
  - PE pooling: pool += E32[c].T @ h_c and esum += E32[c].T @ ones,
    accumulated in PSUM over a macro's 16 chunks -> (32 b-rows, 512)+(32,1).
    fp32r runs the PE at 1 cycle/row (plain fp32 is 4x slower).
  - out = pool * (1/esum) on DVE, DMA out.

h is pre-shuffled on the host to (macro, partition, chunk, d) so each
partition's macro-load is one contiguous 32KB run (the HWDGE sequencer's
per-descriptor cost made the natural layout descriptor-bound).
"""

import numpy as np
import ml_dtypes

# Problem constants (hardcoded: kernel.py must be self-contained)
B, K, D = 4096, 64, 512
N_AGENTS = 64
N_CORES = 8
B_CORE = B // N_CORES            # 512
BK_CORE = B_CORE * K             # 32768
CHUNK = 128                      # bk-slots per chunk (= 2 batch rows)
CHUNKS_PER_MACRO = 16            # chunks per macro (= 32 batch rows)
MACRO_BK = CHUNK * CHUNKS_PER_MACRO   # 2048
NEG_BIG = -1e9

_prog_cache = {}


def _build_program(n_macros: int, repeat: int = 1):
    """Build the SPMD single-core Bass program for a shard of
    n_macros * MACRO_BK (b,k)-slots. repeat>1 wraps the macro loop in a
    device-side For doing the identical (idempotent) work `repeat` times
    (timing only)."""
    import contextlib
    import concourse.bacc as bacc
    import concourse.tile as tile
    import concourse.mybir as mybir

    f32 = mybir.dt.float32
    f32r = mybir.dt.float32r
    bf16 = mybir.dt.bfloat16
    AF = mybir.ActivationFunctionType
    ALU = mybir.AluOpType

    bk = n_macros * MACRO_BK
    b_rows = bk // K
    RPM = MACRO_BK // K   # 32 output rows per macro

    nc = bacc.Bacc("TRN2", target_bir_lowering=False, debug=False,
                   num_devices=N_CORES)

    h_d = nc.dram_tensor("h", [n_macros, CHUNK, CHUNKS_PER_MACRO * D], f32r,
                         kind="ExternalInput").ap()
    oh_d = nc.dram_tensor("oh", [n_macros, N_AGENTS, CHUNKS_PER_MACRO, CHUNK],
                          bf16, kind="ExternalInput").ap()
    pen_d = nc.dram_tensor("pen", [CHUNK, n_macros * CHUNKS_PER_MACRO], f32,
                           kind="ExternalInput").ap()
    keys_d = nc.dram_tensor("keys", [N_AGENTS, D], bf16,
                            kind="ExternalInput").ap()
    ones_d = nc.dram_tensor("ones", [CHUNK, 2], f32r,
                            kind="ExternalInput").ap()
    ez_d = nc.dram_tensor("ez", [CHUNK, 2 * CHUNKS_PER_MACRO], f32r,
                          kind="ExternalInput").ap()
    out_d = nc.dram_tensor("out", [b_rows, D], f32, kind="ExternalOutput").ap()

    with tile.TileContext(nc) as tc:
        with (
            tc.tile_pool(name="const", bufs=1) as const_pool,
            tc.tile_pool(name="h", bufs=3) as h_pool,
            tc.tile_pool(name="oh", bufs=2) as oh_pool,
            tc.tile_pool(name="tmp", bufs=2) as tmp_pool,
            tc.tile_pool(name="logit", bufs=4) as logit_pool,
            tc.tile_pool(name="outp", bufs=2) as out_pool,
            tc.tile_pool(name="recip", bufs=2) as recip_pool,
            tc.tile_pool(name="psw", bufs=3, space="PSUM") as psw,
            tc.tile_pool(name="psp", bufs=2, space="PSUM") as psp,
            tc.tile_pool(name="pse", bufs=2, space="PSUM") as pse,
        ):
            keys_t = const_pool.tile([N_AGENTS, D], bf16)
            nc.sync.dma_start(keys_t[:], keys_d[:])
            ones_t = const_pool.tile([CHUNK, 2], f32r)
            nc.sync.dma_start(ones_t[:], ones_d[:])
            pen_t = const_pool.tile([CHUNK, n_macros * CHUNKS_PER_MACRO], f32)
            nc.scalar.dma_start(pen_t[:], pen_d[:])

            # persistent per-chunk-index stationaries; each buffer's nonzero
            # cells (cols 2c/2c+1) are rewritten by ACT every macro, the rest
            # stay zero forever after this one-time fill
            E32s = []
            for c in range(CHUNKS_PER_MACRO):
                e = const_pool.tile([CHUNK, 2 * CHUNKS_PER_MACRO], f32r,
                                    tag=f"e32_{c}")
                nc.sync.dma_start(e[:], ez_d[:])
                E32s.append(e)

            hd3 = h_d.rearrange("m p (c d) -> m p c d", d=D)
            half = CHUNK // 2  # 64 = K

            rep_ctx = (tc.For_i(0, repeat, 1) if repeat > 1
                       else contextlib.nullcontext())
            with rep_ctx:
                for m in range(n_macros):
                    h_t = h_pool.tile([CHUNK, CHUNKS_PER_MACRO, D], f32r)
                    hc = CHUNKS_PER_MACRO // 2
                    nc.sync.dma_start(h_t[:, 0:hc, :], hd3[m][:, 0:hc, :])
                    nc.sync.dma_start(h_t[:, hc:, :], hd3[m][:, hc:, :])
                    oh_t = oh_pool.tile([N_AGENTS, CHUNKS_PER_MACRO, CHUNK],
                                        bf16)
                    nc.scalar.dma_start(oh_t[:], oh_d[m])

                    pool_ps = psp.tile([RPM, D], f32)
                    esum_ps = pse.tile([RPM, 2], f32)

                    for c in range(CHUNKS_PER_MACRO):
                        # w[bk,:] = keys[sigma[bk]] via one-hot matmul
                        w_ps = psw.tile([CHUNK, D], f32)
                        nc.tensor.matmul(w_ps[:], oh_t[:, c, :], keys_t[:],
                                         start=True, stop=True)

                        # logits: fused mult + free-axis reduce on DVE
                        tmp_t = tmp_pool.tile([CHUNK, D], f32)
                        logit_t = logit_pool.tile([CHUNK, 1], f32)
                        nc.vector.scalar_tensor_tensor(
                            tmp_t[:], h_t[:, c, :].bitcast(f32), 1.0, w_ps[:],
                            op0=ALU.mult, op1=ALU.mult,
                            accum_out=logit_t[:],
                        )

                        # e = exp(logit + pen) into the block columns
                        E32 = E32s[c]
                        pc = m * CHUNKS_PER_MACRO + c
                        nc.scalar.activation(
                            E32[0:half, 2 * c:2 * c + 1], logit_t[0:half, :],
                            AF.Exp, bias=pen_t[0:half, pc:pc + 1], scale=1.0)
                        nc.scalar.activation(
                            E32[half:CHUNK, 2 * c + 1:2 * c + 2],
                            logit_t[half:CHUNK, :],
                            AF.Exp, bias=pen_t[half:CHUNK, pc:pc + 1],
                            scale=1.0)

                        # pool += E32c.T @ h_c ; esum += E32c.T @ ones
                        first, last = (c == 0), (c == CHUNKS_PER_MACRO - 1)
                        nc.tensor.matmul(pool_ps[:], E32[:], h_t[:, c, :],
                                         start=first, stop=last)
                        nc.tensor.matmul(esum_ps[:], E32[:], ones_t[:],
                                         start=first, stop=last)

                    recip_t = recip_pool.tile([RPM, 1], f32)
                    nc.vector.reciprocal(recip_t[:], esum_ps[:, 0:1])
                    out_t = out_pool.tile([RPM, D], f32)
                    nc.vector.tensor_scalar_mul(out_t[:], pool_ps[:],
                                                recip_t[:])
                    nc.scalar.dma_start(out_d[m * RPM:(m + 1) * RPM, :],
                                        out_t[:])

    nc.compile()
    return nc


def get_program(n_macros: int = B_CORE * K // MACRO_BK):
    if n_macros not in _prog_cache:
        _prog_cache[n_macros] = _build_program(n_macros)
    return _prog_cache[n_macros]


def _build_program_repeat(n_macros: int, repeat: int):
    return _build_program(n_macros, repeat=repeat)


def prep_core_inputs(h_bk: np.ndarray, sigma_bk: np.ndarray,
                     keys_bf16: np.ndarray):
    """Host-side prep of one core's input map.
    h_bk: (bk, D) float32, sigma_bk: (bk,) int."""
    bk = h_bk.shape[0]
    n_macros = bk // MACRO_BK
    sig = sigma_bk.astype(np.int64)
    # one-hot (a == sigma); sigma == N_AGENTS (unassigned) matches nothing
    oh = (sig[None, :] == np.arange(N_AGENTS, dtype=np.int64)[:, None])
    oh = oh.astype(ml_dtypes.bfloat16)          # (A, bk)
    oh = oh.reshape(N_AGENTS, n_macros, CHUNKS_PER_MACRO, CHUNK)
    oh = np.ascontiguousarray(oh.transpose(1, 0, 2, 3))  # (m, A, c, j)

    pen = np.where(sig < N_AGENTS, np.float32(0.0), np.float32(NEG_BIG))
    pen = pen.reshape(n_macros, CHUNKS_PER_MACRO, CHUNK)
    pen = np.ascontiguousarray(pen.transpose(2, 0, 1)).astype(np.float32)
    pen = pen.reshape(CHUNK, n_macros * CHUNKS_PER_MACRO)

    h_shuf = np.ascontiguousarray(
        h_bk.reshape(n_macros, CHUNKS_PER_MACRO, CHUNK, D)
            .transpose(0, 2, 1, 3), dtype=np.float32)
    h_shuf = h_shuf.reshape(n_macros, CHUNK, CHUNKS_PER_MACRO * D)

    return {
        "h": h_shuf,
        "oh": oh,
        "pen": pen,
        "keys": keys_bf16,
        "ones": np.ones((CHUNK, 2), dtype=np.float32),
        "ez": np.zeros((CHUNK, 2 * CHUNKS_PER_MACRO), dtype=np.float32),
    }


def kernel(h, keys, sigma):
    from concourse.bass_utils import run_bass_kernel_spmd

    h = np.asarray(h, dtype=np.float32)
    keys = np.asarray(keys, dtype=np.float32)
    sigma = np.asarray(sigma)

    keys_bf16 = keys.astype(ml_dtypes.bfloat16)
    h2 = h.reshape(B * K, D)
    sig2 = sigma.reshape(B * K)

    in_maps = []
    for i in range(N_CORES):
        lo, hi = i * BK_CORE, (i + 1) * BK_CORE
        in_maps.append(prep_core_inputs(h2[lo:hi], sig2[lo:hi], keys_bf16))

    nc = get_program()
    res = run_bass_kernel_spmd(nc, in_maps, list(range(N_CORES)))
    out = np.concatenate([res.results[i]["out"] for i in range(N_CORES)],
                         axis=0)
    return out.astype(np.float32)


if __name__ == "__main__":
    rng = np.random.default_rng(0)
    h = rng.standard_normal((B, K, D), dtype=np.float32)
    keys = (rng.standard_normal((N_AGENTS, D), dtype=np.float32) * 0.01)
    sigma = rng.integers(0, N_AGENTS + 1, size=(B, K)).astype(np.int32)
    out = kernel(h=h, keys=keys, sigma=sigma)
    print("out", out.shape, out.dtype, float(np.abs(out).mean()))
